# revision 1
# baseline (speedup 1.0000x reference)
"""Trainium2 Bass kernel for nn_DCell (hierarchical DCell-style GNN).

Sharding: subsystem-parallel across 8 NeuronCores. Each core owns 64 of the
512 leaf subsystems (16 groups of 4 leaves, block-diagonal matmuls with
K=128) and the 4 mid subsystems fed by exactly those leaves. BatchNorm batch
stats (full batch B=2048) are therefore fully local to a core for the leaf
and mid layers; the BN affine is folded into the *next* layer's weights so
no full-size normalization pass over activations is ever needed. The root
layer is computed as per-core partial pre-activations (each core contributes
its 4 mids' features + a 16-row slice of the root gene input) that are
summed with a single AllReduce; every core then redundantly finishes the
root (tanh + full-batch BN) on the small [38, 2048] tensor.

Compute dtype is bf16 (inputs cast on host -> half the HBM traffic, full
TensorE rate); all normalization statistics and folds are fp32.

kernel(**inputs) takes full unsharded inputs, returns the full [2048, 38]
float32 output.
"""

import ml_dtypes
import numpy as np

import concourse.bass as bass
import concourse.mybir as mybir
import concourse.tile as tile
from concourse import bacc
from concourse import bass_utils

# Problem constants (hardcoded; kernel.py must be self-contained)
S, B, GL, OL = 512, 2048, 32, 20
M, C, GM, OM = 32, 16, 64, 20
GR, OR = 128, 38
EPS = 1e-5
NCORES = 8
LPC = S // NCORES      # 64 leaves per core
GPC = LPC // 4         # 16 leaf groups of 4 per core
MPC = M // NCORES      # 4 mids per core
BT = 512               # batch tile (free dim per matmul / psum bank)
NBT = B // BT          # 4

f32 = mybir.dt.float32
bf16 = mybir.dt.bfloat16
i32 = mybir.dt.int32
AF = mybir.ActivationFunctionType
ALU = mybir.AluOpType
NPBF16 = ml_dtypes.bfloat16

MAGIC = 0x5F3759DF  # fast inverse sqrt seed


def _emit_rsqrt(nc, sp, tag, out, a, magic_t, n):
    """out = 1/sqrt(a) elementwise on [P, n] fp32 tiles, DVE-only.

    Quake magic seed + 2 Newton iterations: rel err ~5e-6. a > 0.
    """
    P = a.shape[0]
    sh = sp.tile([P, n], i32, tag=f"{tag}sh", name=f"{tag}sh")
    nc.vector.tensor_scalar(sh, a.bitcast(i32), 1, None,
                            ALU.arith_shift_right)
    y0 = sp.tile([P, n], i32, tag=f"{tag}y0", name=f"{tag}y0")
    nc.vector.tensor_tensor(y0, magic_t[:P, 0:n], sh, ALU.subtract)
    y = y0.bitcast(f32)
    for it in range(2):
        # y <- y * (1.5 - 0.5*a*y*y)
        t1 = sp.tile([P, n], f32, tag=f"{tag}t1", name=f"{tag}t1_{it}")
        nc.vector.tensor_mul(t1, y, y)                    # y^2
        t2 = sp.tile([P, n], f32, tag=f"{tag}t2", name=f"{tag}t2_{it}")
        nc.vector.tensor_mul(t2, a, t1)                   # a*y^2
        t3 = sp.tile([P, n], f32, tag=f"{tag}t3", name=f"{tag}t3_{it}")
        nc.vector.tensor_scalar(t3, t2, -0.5, 1.5, ALU.mult, ALU.add)
        dst = out if it == 1 else sp.tile([P, n], f32, tag=f"{tag}y",
                                          name=f"{tag}y_{it}")
        nc.vector.tensor_mul(dst, y, t3)
        y = dst


def _build_nc():
    """Build (once) the SPMD Bass program run identically on all 8 cores."""
    nc = bacc.Bacc(
        "TRN2",
        target_bir_lowering=False,
        debug=False,
        enable_asserts=False,
        num_devices=NCORES,
    )

    # ---- per-core external I/O ----
    xleaf = nc.dram_tensor("xleaf", [GPC, 128, B], bf16, kind="ExternalInput").ap()
    wleaf = nc.dram_tensor("wleaf", [128, GPC * 80], bf16, kind="ExternalInput").ap()
    bleaf = nc.dram_tensor("bleaf", [80, GPC], f32, kind="ExternalInput").ap()
    gleaf = nc.dram_tensor("gleaf", [80, GPC], f32, kind="ExternalInput").ap()
    beleaf = nc.dram_tensor("beleaf", [80, GPC], f32, kind="ExternalInput").ap()
    xmid2 = nc.dram_tensor("xmid2", [2, 128, B], bf16, kind="ExternalInput").ap()
    wgmid = nc.dram_tensor("wgmid", [80, GPC * 80], bf16, kind="ExternalInput").ap()
    wxmid2 = nc.dram_tensor("wxmid2", [128, 2 * 80], bf16, kind="ExternalInput").ap()
    bmid = nc.dram_tensor("bmid", [80, 1], f32, kind="ExternalInput").ap()
    gmid = nc.dram_tensor("gmid", [80, 1], f32, kind="ExternalInput").ap()
    bemid = nc.dram_tensor("bemid", [80, 1], f32, kind="ExternalInput").ap()
    wcroot = nc.dram_tensor("wcroot", [80, OR], bf16, kind="ExternalInput").ap()
    wgroot = nc.dram_tensor("wgroot", [16, OR], bf16, kind="ExternalInput").ap()
    xroot = nc.dram_tensor("xroot", [16, B], bf16, kind="ExternalInput").ap()
    broot = nc.dram_tensor("broot", [OR, 1], f32, kind="ExternalInput").ap()
    groot = nc.dram_tensor("groot", [OR, 1], f32, kind="ExternalInput").ap()
    beroot = nc.dram_tensor("beroot", [OR, 1], f32, kind="ExternalInput").ap()
    y = nc.dram_tensor("y", [OR, B], f32, kind="ExternalOutput").ap()

    with tile.TileContext(nc) as tc:
        with (
            tc.tile_pool(name="const", bufs=1) as cp,
            tc.tile_pool(name="xp", bufs=8) as xp,
            tc.tile_pool(name="lt", bufs=16) as ltp,
            tc.tile_pool(name="small", bufs=2) as sp,
            tc.tile_pool(name="big", bufs=1) as bp,
            tc.tile_pool(name="psA", bufs=2, space="PSUM") as psA,
            tc.tile_pool(name="psM", bufs=4, space="PSUM") as psM,
            tc.tile_pool(name="dram", bufs=1, space="DRAM") as dp,
        ):
            # ---- load constants/weights into SBUF ----
            wleaf_sb = cp.tile_from(wleaf)
            bleaf_sb = cp.tile_from(bleaf)
            gleaf_sb = cp.tile_from(gleaf)
            beleaf_sb = cp.tile_from(beleaf)
            wgmid_sb = cp.tile_from(wgmid)
            wxmid2_sb = cp.tile_from(wxmid2)
            xmid2a_sb = cp.tile_from(xmid2[0])
            xmid2b_sb = cp.tile_from(xmid2[1])
            xmid2_sb = [xmid2a_sb, xmid2b_sb]
            bmid_sb = cp.tile_from(bmid)
            gmid_sb = cp.tile_from(gmid)
            bemid_sb = cp.tile_from(bemid)
            wcroot_sb = cp.tile_from(wcroot)
            wgroot_sb = cp.tile_from(wgroot)
            xroot_sb = cp.tile_from(xroot)
            broot_sb = cp.tile_from(broot)
            groot_sb = cp.tile_from(groot)
            beroot_sb = cp.tile_from(beroot)

            magic_t = cp.tile([80, 4], i32, tag="magic", name="magict")
            nc.vector.memset(magic_t, MAGIC)
            zbias = cp.tile([80, 1], f32, tag="zbias", name="zbias")
            nc.vector.memset(zbias, 0.0)

            # persistent mid-accumulation psum banks (one per batch tile)
            mid_ps = [psM.tile([80, BT], f32, tag="mid", name=f"midps{b}")
                      for b in range(NBT)]

            lt_tiles = []
            mv_cols = cp.tile([80, GPC, 2], f32, tag="mvall", name="mvall")
            stats_t = None
            for gi in range(GPC):
                mi, gj = gi // 4, gi % 4
                act_stats = (gi >= 8)       # later groups: stats via ACT accum
                ltile = ltp.tile([80, B], bf16, tag="lt", name=f"lt{gi}")
                lt_tiles.append(ltile)
                if gj == 0 and gi < 8:
                    stats_t = sp.tile([80, 4, NBT, 6], f32, tag="st",
                                      name=f"st{mi}")
                if act_stats:
                    sumx = sp.tile([80, 2], f32, tag="sx", name=f"sx{gi}")
                    sumq = sp.tile([80, 2], f32, tag="sq", name=f"sq{gi}")
                # ---- leaf matmul + tanh (+ Sum accum) per half batch ----
                xt = xp.tile([128, B], bf16, tag="x", name=f"x{gi}")
                nc.sync.dma_start(out=xt, in_=xleaf[gi])
                for h in range(2):
                    ps = psA.tile([80, 2 * BT], f32, tag="leaf",
                                  name=f"lfps{gi}_{h}")
                    for s2 in range(2):
                        nc.tensor.matmul(
                            ps[:, s2 * BT:(s2 + 1) * BT],
                            wleaf_sb[:, 80 * gi:80 * gi + 80],
                            xt[:, (2 * h + s2) * BT:(2 * h + s2 + 1) * BT],
                            start=True, stop=True)
                    nc.scalar.activation(
                        ltile[:, 2 * h * BT:2 * (h + 1) * BT], ps[:, :],
                        AF.Tanh, bias=bleaf_sb[:, gi:gi + 1], scale=1.0,
                        accum_out=sumx[:, h:h + 1] if act_stats else None)
                # ---- leaf BN stats for this group ----
                if act_stats:
                    sqs = sp.tile([80, 2 * BT], bf16, tag="sqs",
                                  name=f"sqs{gi}")
                    for h in range(2):
                        nc.scalar.activation(
                            sqs[:, :], ltile[:, 2 * h * BT:2 * (h + 1) * BT],
                            AF.Square, bias=zbias[:, 0:1],
                            accum_out=sumq[:, h:h + 1])
                    # mean = sum(sumx)/B ; var = sum(sumq)/B - mean^2
                    sxr = sp.tile([80, 1], f32, tag="sxr", name=f"sxr{gi}")
                    nc.vector.tensor_reduce(
                        out=sxr, in_=sumx[:, :],
                        op=ALU.add, axis=mybir.AxisListType.X)
                    nc.vector.tensor_scalar_mul(
                        mv_cols[:, gi, 0:1], sxr, 1.0 / B)
                    mq = sp.tile([80, 1], f32, tag="mq2", name=f"mq2{gi}")
                    nc.vector.tensor_mul(mq, mv_cols[:, gi, 0:1],
                                         mv_cols[:, gi, 0:1])
                    sqr = sp.tile([80, 1], f32, tag="sqr", name=f"sqr{gi}")
                    nc.vector.tensor_reduce(
                        out=sqr, in_=sumq[:, :],
                        op=ALU.add, axis=mybir.AxisListType.X)
                    nc.vector.tensor_scalar(
                        mv_cols[:, gi, 1:2], sqr, 1.0 / B, mq,
                        ALU.mult, ALU.subtract)
                else:
                    for bt in range(NBT):
                        nc.vector.bn_stats(
                            out=stats_t[:, gj, bt, :],
                            in_=ltile[:, bt * BT:(bt + 1) * BT])
                    nc.vector.bn_aggr(out=mv_cols[:, gi, :],
                                      in_=stats_t[:, gj, :, :])

                if gj != 3:
                    continue
                # ---- mid mi: fold BN into child weights, accumulate ----
                mean4 = mv_cols[:, 4 * mi:4 * mi + 4, 0]
                var4 = mv_cols[:, 4 * mi:4 * mi + 4, 1]
                a4 = sp.tile([80, 4], f32, tag="a4", name=f"a4{mi}")
                nc.vector.tensor_scalar_add(a4, var4, EPS)
                rs4 = sp.tile([80, 4], f32, tag="rs4", name=f"rs4{mi}")
                _emit_rsqrt(nc, sp, "lf", rs4, a4, magic_t, 4)
                s_t = sp.tile([80, 4], f32, tag="s", name=f"s{mi}")
                nc.vector.tensor_mul(s_t, gleaf_sb[:, 4 * mi:4 * mi + 4], rs4)
                ms = sp.tile([80, 4], f32, tag="ms", name=f"ms{mi}")
                nc.vector.tensor_mul(ms, mean4, s_t)
                t_t = sp.tile([80, 4], f32, tag="t", name=f"t{mi}")
                nc.vector.tensor_sub(t_t, beleaf_sb[:, 4 * mi:4 * mi + 4], ms)

                # apply BN in place on the 4 bf16 leaf tiles (4x DVE mode)
                for g2 in range(4):
                    idx = 4 * mi + g2
                    nc.vector.tensor_scalar(
                        lt_tiles[idx][:, :], lt_tiles[idx][:, :],
                        s_t[:, g2:g2 + 1], t_t[:, g2:g2 + 1],
                        ALU.mult, ALU.add)
                for bt in range(NBT):
                    for g2 in range(4):
                        idx = 4 * mi + g2
                        nc.tensor.matmul(
                            mid_ps[bt][:, :],
                            wgmid_sb[:, 80 * idx:80 * idx + 80],
                            lt_tiles[idx][:, bt * BT:(bt + 1) * BT],
                            start=(mi == 0 and g2 == 0), stop=False)
                    if mi % 2 == 1:  # gene blocks for mid pair (mi-1, mi)
                        pr = mi // 2
                        nc.tensor.matmul(
                            mid_ps[bt][:, :],
                            wxmid2_sb[:, 80 * pr:80 * pr + 80],
                            xmid2_sb[pr][:, bt * BT:(bt + 1) * BT],
                            start=False, stop=(mi == 3))

            # ---- mid finish: tanh, BN stats ----
            tmt = bp.tile([80, B], bf16, tag="tm", name="tmt")
            mst = sp.tile([80, NBT, 6], f32, tag="mst", name="mst")
            for bt in range(NBT):
                nc.scalar.activation(
                    tmt[:, bt * BT:(bt + 1) * BT], mid_ps[bt][:, :],
                    AF.Tanh, bias=bmid_sb[:, 0:1], scale=1.0)
                nc.vector.bn_stats(out=mst[:, bt, :],
                                   in_=tmt[:, bt * BT:(bt + 1) * BT])
            mmv = sp.tile([80, 2], f32, tag="mmv", name="mmv")
            nc.vector.bn_aggr(out=mmv[:, :], in_=mst[:, :, :])
            ma = sp.tile([80, 1], f32, tag="ma", name="ma")
            nc.vector.tensor_scalar_add(ma, mmv[:, 1:2], EPS)
            mrs = sp.tile([80, 1], f32, tag="mrs", name="mrs")
            _emit_rsqrt(nc, sp, "md", mrs, ma, magic_t, 1)
            msm = sp.tile([80, 1], f32, tag="msm", name="msm")
            nc.vector.tensor_mul(msm, gmid_sb[:, :], mrs)
            mms = sp.tile([80, 1], f32, tag="mms", name="mms")
            nc.vector.tensor_mul(mms, mmv[:, 0:1], msm)
            mtm = sp.tile([80, 1], f32, tag="mtm", name="mtm")
            nc.vector.tensor_sub(mtm, bemid_sb[:, :], mms)
            # apply mid BN in place on tmt
            nc.vector.tensor_scalar(tmt[:, :], tmt[:, :], msm[:, 0:1],
                                    mtm[:, 0:1], ALU.mult, ALU.add)

            # ---- root partial pre-activation [38, B] ----
            partial = bp.tile([OR, B], bf16, tag="prt", name="partial")
            for bt in range(NBT):
                psr = psA.tile([OR, BT], f32, tag="leaf", name=f"rtps{bt}")
                nc.tensor.matmul(
                    psr[:, :], wcroot_sb[:, :],
                    tmt[:, bt * BT:(bt + 1) * BT],
                    start=True, stop=False)
                nc.tensor.matmul(
                    psr[:, :], wgroot_sb[:, :],
                    xroot_sb[:, bt * BT:(bt + 1) * BT],
                    start=False, stop=True)
                nc.vector.tensor_copy(
                    partial[:, bt * BT:(bt + 1) * BT], psr[:, :])

            # ---- AllReduce the partial root pre-activation (bf16) ----
            cc_in = dp.tile([OR, B], bf16, tag="ccin", name="ccin")
            cc_out = dp.tile([OR, B], bf16, tag="ccout", name="ccout")
            nc.sync.dma_start(out=cc_in[:, :], in_=partial[:, :])
            nc.gpsimd.collective_compute(
                "AllReduce",
                ALU.add,
                replica_groups=[list(range(NCORES))],
                ins=[cc_in.opt()],
                outs=[cc_out.opt()],
            )
            rsum = bp.tile([OR, B], bf16, tag="prt2", name="rsum")
            nc.sync.dma_start(out=rsum[:, :], in_=cc_out[:, :])

            # ---- root finish: tanh, full-batch BN, write out ----
            rt = bp.tile([OR, B], bf16, tag="rt", name="rt")
            nc.scalar.activation(rt[:, :], rsum[:, :], AF.Tanh,
                                 bias=broot_sb[:, 0:1], scale=1.0)
            rst = sp.tile([OR, NBT, 6], f32, tag="rst", name="rst")
            for bt in range(NBT):
                nc.vector.bn_stats(out=rst[:, bt, :],
                                   in_=rt[:, bt * BT:(bt + 1) * BT])
            rmv = sp.tile([OR, 2], f32, tag="rmv", name="rmv")
            nc.vector.bn_aggr(out=rmv[:, :], in_=rst[:, :, :])
            ra = sp.tile([OR, 1], f32, tag="ra", name="ra")
            nc.vector.tensor_scalar_add(ra, rmv[:, 1:2], EPS)
            rrs = sp.tile([OR, 1], f32, tag="rrs", name="rrs")
            _emit_rsqrt(nc, sp, "rt", rrs, ra, magic_t, 1)
            rsc = sp.tile([OR, 1], f32, tag="rsc", name="rsc")
            nc.vector.tensor_mul(rsc, groot_sb[:, :], rrs)
            rms = sp.tile([OR, 1], f32, tag="rms", name="rms")
            nc.vector.tensor_mul(rms, rmv[:, 0:1], rsc)
            rsh = sp.tile([OR, 1], f32, tag="rsh", name="rsh")
            nc.vector.tensor_sub(rsh, beroot_sb[:, :], rms)
            ysb = bp.tile([OR, B], f32, tag="ysb", name="ysb")
            nc.vector.tensor_scalar(ysb[:, :], rt[:, :], rsc[:, 0:1],
                                    rsh[:, 0:1], ALU.mult, ALU.add)
            nc.sync.dma_start(out=y, in_=ysb[:, :])

    nc.compile()
    return nc


def _prep_in_maps(inputs):
    """Host-side sharding + layout prep (incl. bf16 cast). 8 in_maps."""
    f = np.float32
    x_leaf = np.asarray(inputs["x_leaf"], dtype=f)
    x_mid = np.asarray(inputs["x_mid"], dtype=f)
    x_root = np.asarray(inputs["x_root"], dtype=f)
    W_leaf = np.asarray(inputs["W_leaf"], dtype=f)
    b_leaf = np.asarray(inputs["b_leaf"], dtype=f)
    g_leaf = np.asarray(inputs["g_leaf"], dtype=f)
    be_leaf = np.asarray(inputs["be_leaf"], dtype=f)
    W_mid = np.asarray(inputs["W_mid"], dtype=f)
    b_mid = np.asarray(inputs["b_mid"], dtype=f)
    g_mid = np.asarray(inputs["g_mid"], dtype=f)
    be_mid = np.asarray(inputs["be_mid"], dtype=f)
    W_root = np.asarray(inputs["W_root"], dtype=f)
    b_root = np.asarray(inputs["b_root"], dtype=f)
    g_root = np.asarray(inputs["g_root"], dtype=f)
    be_root = np.asarray(inputs["be_root"], dtype=f)

    # gene-major leaf inputs, 4 leaves stacked per 128-partition group
    xleafT = np.ascontiguousarray(
        x_leaf.reshape(NCORES, GPC, 4, B, GL).transpose(0, 1, 2, 4, 3)
        .reshape(NCORES, GPC, 128, B)).astype(NPBF16)
    # mid gene inputs: per core, mid pairs (0,1) and (2,3) stacked to 128
    xmidT = (x_mid.reshape(NCORES, 2, 2, B, GM).transpose(0, 1, 2, 4, 3)
             .reshape(NCORES, 2, 128, B)).astype(NPBF16)
    xrootT = np.ascontiguousarray(x_root.T).astype(NPBF16)     # [128, B]

    in_maps = []
    for c in range(NCORES):
        d = {}
        d["xleaf"] = np.ascontiguousarray(xleafT[c])
        # block-diagonal leaf weights [128, 16*80]
        wl = np.zeros((128, GPC * 80), f)
        for gi in range(GPC):
            for j in range(4):
                s = LPC * c + 4 * gi + j
                wl[32 * j:32 * j + 32,
                   80 * gi + 20 * j:80 * gi + 20 * j + 20] = W_leaf[s]
        d["wleaf"] = wl.astype(NPBF16)
        for src, name in ((b_leaf, "bleaf"), (g_leaf, "gleaf"),
                          (be_leaf, "beleaf")):
            d[name] = np.ascontiguousarray(
                src[LPC * c:LPC * (c + 1)].reshape(GPC, 80).T)
        d["xmid2"] = np.ascontiguousarray(xmidT[c])
        wg = np.zeros((80, GPC * 80), f)
        # gene blocks for mid pairs: [128, 2*80]
        wx2 = np.zeros((128, 2 * 80), f)
        for mi in range(MPC):
            m = MPC * c + mi
            for gj in range(4):
                idx = 4 * mi + gj
                wg[:, 80 * idx + 20 * mi:80 * idx + 20 * mi + 20] = \
                    W_mid[m, GM + 80 * gj:GM + 80 * gj + 80, :]
            pr, sub = mi // 2, mi % 2
            wx2[64 * sub:64 * sub + 64,
                80 * pr + 20 * mi:80 * pr + 20 * mi + 20] = W_mid[m, :GM, :]
        d["wgmid"] = wg.astype(NPBF16)
        d["wxmid2"] = wx2.astype(NPBF16)
        for src, name in ((b_mid, "bmid"), (g_mid, "gmid"), (be_mid, "bemid")):
            d[name] = np.ascontiguousarray(
                src[MPC * c:MPC * (c + 1)].reshape(80, 1))
        d["wcroot"] = np.ascontiguousarray(
            W_root[GR + 80 * c:GR + 80 * (c + 1), :]).astype(NPBF16)
        d["wgroot"] = np.ascontiguousarray(
            W_root[16 * c:16 * (c + 1), :]).astype(NPBF16)
        d["xroot"] = np.ascontiguousarray(xrootT[16 * c:16 * (c + 1), :])
        for src, name in ((b_root, "broot"), (g_root, "groot"),
                          (be_root, "beroot")):
            d[name] = np.ascontiguousarray(src.reshape(OR, 1))
        in_maps.append(d)
    return in_maps


_NC_CACHE = {}


def _get_nc():
    if "nc" not in _NC_CACHE:
        _NC_CACHE["nc"] = _build_nc()
    return _NC_CACHE["nc"]


def kernel(**inputs) -> np.ndarray:
    nc = _get_nc()
    in_maps = _prep_in_maps(inputs)
    res = bass_utils.run_bass_kernel_spmd(
        nc, in_maps, core_ids=list(range(NCORES)))
    out = res.results[0]["y"]                                   # [38, 2048]
    return np.ascontiguousarray(out.T).astype(np.float32)       # [2048, 38]



# revision 10
# speedup vs baseline: 1.0977x; 1.0977x over previous
"""Trainium2 Bass kernel for nn_DCell (hierarchical DCell-style GNN).

Sharding: subsystem-parallel across 8 NeuronCores. Each core owns 64 of the
512 leaf subsystems (16 groups of 4 leaves, block-diagonal matmuls with
K=128) and the 4 mid subsystems fed by exactly those leaves. BatchNorm batch
stats are fully local for leaf and mid layers. The mid BN affine is folded
into the root weights; each core computes its root partial pre-activation in
a transposed [batch-on-partitions] wire layout [128, 16*38], which is summed
across cores with one AllReduce. Every core then redundantly finishes the
root (tanh + full-batch BN via a gpsimd partition-reduce) and writes the
[128, 608] f32 result; the host reassembles [2048, 38].

Work is spread across all four compute engines: PE (matmuls), ACT (tanh +
some stats via Square-accumulate), DVE (bn_stats, folds, finish), Pool/
GpSimd (BN applies, small stat reductions, partition reduce/broadcast).

kernel(**inputs) takes full unsharded inputs, returns [2048, 38] float32.
"""

import ml_dtypes
import numpy as np

import concourse.bass as bass
import concourse.mybir as mybir
import concourse.tile as tile
from concourse import bacc
from concourse import bass_utils

# Problem constants (hardcoded; kernel.py must be self-contained)
S, B, GL, OL = 512, 2048, 32, 20
M, C, GM, OM = 32, 16, 64, 20
GR, OR = 128, 38
EPS = 1e-5
NCORES = 8
LPC = S // NCORES      # 64 leaves per core
GPC = LPC // 4         # 16 leaf groups of 4 per core
MPC = M // NCORES      # 4 mids per core
BT = 512               # batch tile (free dim per matmul / psum bank)
NBT = B // BT          # 4
NCH = B // 128         # 16 batch chunks of 128 (wire layout)

f32 = mybir.dt.float32
bf16 = mybir.dt.bfloat16
i32 = mybir.dt.int32
AF = mybir.ActivationFunctionType
ALU = mybir.AluOpType
AX = mybir.AxisListType
NPBF16 = ml_dtypes.bfloat16

MAGIC = 0x5F3759DF  # fast inverse sqrt seed

# stats-engine assignment per leaf group: these groups compute sumsq via
# ACT Square+accumulate, the rest via DVE bn_stats
ACT_STATS = {4, 5, 6, 7, 8, 9, 10, 11}


def _emit_rsqrt(nc, eng, sp, tag, out, a, magic_t, n):
    """out = 1/sqrt(a) elementwise on [P, n] fp32 tiles, DVE/Pool-only.

    Quake magic seed + 2 Newton iterations: rel err ~5e-6. a > 0.
    """
    P = a.shape[0]
    sh = sp.tile([P, n], i32, tag=f"{tag}sh", name=f"{tag}sh")
    eng.tensor_scalar(sh, a.bitcast(i32), 1, None, ALU.arith_shift_right)
    y0 = sp.tile([P, n], i32, tag=f"{tag}y0", name=f"{tag}y0")
    eng.tensor_tensor(y0, magic_t[:P, 0:n], sh, ALU.subtract)
    y = y0.bitcast(f32)
    for it in range(2):
        # y <- y * (1.5 - 0.5*a*y*y)
        t1 = sp.tile([P, n], f32, tag=f"{tag}t1", name=f"{tag}t1_{it}")
        eng.tensor_tensor(t1, y, y, ALU.mult)                # y^2
        t2 = sp.tile([P, n], f32, tag=f"{tag}t2", name=f"{tag}t2_{it}")
        eng.tensor_tensor(t2, a, t1, ALU.mult)               # a*y^2
        t3 = sp.tile([P, n], f32, tag=f"{tag}t3", name=f"{tag}t3_{it}")
        eng.tensor_scalar(t3, t2, -0.5, 1.5, ALU.mult, ALU.add)
        dst = out if it == 1 else sp.tile([P, n], f32, tag=f"{tag}y",
                                          name=f"{tag}y_{it}")
        eng.tensor_tensor(dst, y, t3, ALU.mult)
        y = dst


def _build_nc():
    """Build (once) the SPMD Bass program run identically on all 8 cores."""
    nc = bacc.Bacc(
        "TRN2",
        target_bir_lowering=False,
        debug=False,
        enable_asserts=False,
        num_devices=NCORES,
    )

    # ---- per-core external I/O ----
    xleaf = nc.dram_tensor("xleaf", [GPC, 128, B], bf16, kind="ExternalInput").ap()
    wleaf = nc.dram_tensor("wleaf", [128, GPC * 80], bf16, kind="ExternalInput").ap()
    bleaf = nc.dram_tensor("bleaf", [80, GPC], f32, kind="ExternalInput").ap()
    gleaf = nc.dram_tensor("gleaf", [80, GPC], f32, kind="ExternalInput").ap()
    beleaf = nc.dram_tensor("beleaf", [80, GPC], f32, kind="ExternalInput").ap()
    xmid2 = nc.dram_tensor("xmid2", [2, 128, B], bf16, kind="ExternalInput").ap()
    wgmid = nc.dram_tensor("wgmid", [80, GPC * 80], bf16, kind="ExternalInput").ap()
    wxmid2 = nc.dram_tensor("wxmid2", [128, 2 * 80], bf16, kind="ExternalInput").ap()
    bmid = nc.dram_tensor("bmid", [80, 1], f32, kind="ExternalInput").ap()
    gmid = nc.dram_tensor("gmid", [80, 1], f32, kind="ExternalInput").ap()
    bemid = nc.dram_tensor("bemid", [80, 1], f32, kind="ExternalInput").ap()
    wcroot = nc.dram_tensor("wcroot", [80, OR], bf16, kind="ExternalInput").ap()
    wgroot = nc.dram_tensor("wgroot", [16, OR], bf16, kind="ExternalInput").ap()
    xroot17 = nc.dram_tensor("xroot17", [17, B], bf16, kind="ExternalInput").ap()
    broot8 = nc.dram_tensor("broot8", [1, OR], f32, kind="ExternalInput").ap()
    grootb = nc.dram_tensor("grootb", [128, OR], f32, kind="ExternalInput").ap()
    berootb = nc.dram_tensor("berootb", [128, OR], f32, kind="ExternalInput").ap()
    y = nc.dram_tensor("y", [128, NCH * OR], f32, kind="ExternalOutput").ap()

    with tile.TileContext(nc) as tc:
        with (
            tc.tile_pool(name="const", bufs=1) as cp,
            tc.tile_pool(name="xp", bufs=3) as xp,
            tc.tile_pool(name="lt", bufs=16) as ltp,
            tc.tile_pool(name="small", bufs=2) as sp,
            tc.tile_pool(name="big", bufs=1) as bp,
            tc.tile_pool(name="psA", bufs=2, space="PSUM") as psA,
            tc.tile_pool(name="psM", bufs=4, space="PSUM") as psM,
            tc.tile_pool(name="dram", bufs=1, space="DRAM") as dp,
        ):
            # ---- load constants/weights into SBUF (xmid2 deferred) ----
            wleaf_sb = cp.tile_from(wleaf)
            bleaf_sb = cp.tile_from(bleaf)
            gleaf_sb = cp.tile_from(gleaf)
            beleaf_sb = cp.tile_from(beleaf)
            wgmid_sb = cp.tile_from(wgmid)
            wxmid2_sb = cp.tile_from(wxmid2)
            bmid_sb = cp.tile_from(bmid)
            gmid_sb = cp.tile_from(gmid)
            bemid_sb = cp.tile_from(bemid)
            wcroot_sb = cp.tile_from(wcroot)
            broot8_sb = cp.tile_from(broot8)
            grootb_sb = cp.tile_from(grootb)
            berootb_sb = cp.tile_from(berootb)

            # stacked root lhsT: rows 0-79 mid tanh out, 80-96 xroot+ones
            stack = bp.tile([97, B], bf16, tag="stack", name="stack")
            nc.sync.dma_start(out=stack[80:97, :], in_=xroot17)
            # stacked root weights: 0-79 scaled wcroot, 80-95 wgroot,
            # 96 = t_mid fold + b_root/8
            wrt = bp.tile([97, OR], bf16, tag="wrt", name="wrt")
            nc.sync.dma_start(out=wrt[80:96, :], in_=wgroot)

            magic_t = cp.tile([80, 4], i32, tag="magic", name="magict")
            nc.vector.memset(magic_t, MAGIC)
            magic_r = cp.tile([128, OR], i32, tag="magicr", name="magicr")
            nc.vector.memset(magic_r, MAGIC)
            zbias = cp.tile([80, 1], f32, tag="zbias", name="zbias")
            nc.vector.memset(zbias, 0.0)

            # persistent mid-accumulation psum banks (one per batch tile)
            mid_ps = [psM.tile([80, BT], f32, tag="mid", name=f"midps{b}")
                      for b in range(NBT)]

            lt_tiles = []
            mv_cols = cp.tile([80, GPC, 2], f32, tag="mvall", name="mvall")
            xmid2_sb = [None, None]
            for gi in range(GPC):
                mi, gj = gi // 4, gi % 4
                act_stats = gi in ACT_STATS
                ltile = ltp.tile([80, B], bf16, tag="lt", name=f"lt{gi}")
                lt_tiles.append(ltile)
                if act_stats:
                    sumx = sp.tile([80, 2], f32, tag="sx", name=f"sx{gi}")
                    sumq = sp.tile([80, 2], f32, tag="sq", name=f"sq{gi}")
                # ---- leaf matmul + tanh (+ Sum accum) per half batch ----
                xt = xp.tile([128, B], bf16, tag="x", name=f"x{gi}")
                for h in range(2):
                    nc.sync.dma_start(
                        out=xt[:, h * 2 * BT:(h + 1) * 2 * BT],
                        in_=xleaf[gi, :, h * 2 * BT:(h + 1) * 2 * BT])
                for h in range(2):
                    ps = psA.tile([80, 2 * BT], f32, tag="leaf",
                                  name=f"lfps{gi}_{h}")
                    for s2 in range(2):
                        nc.tensor.matmul(
                            ps[:, s2 * BT:(s2 + 1) * BT],
                            wleaf_sb[:, 80 * gi:80 * gi + 80],
                            xt[:, (2 * h + s2) * BT:(2 * h + s2 + 1) * BT],
                            start=True, stop=True)
                    nc.scalar.activation(
                        ltile[:, 2 * h * BT:2 * (h + 1) * BT], ps[:, :],
                        AF.Tanh, bias=bleaf_sb[:, gi:gi + 1], scale=1.0,
                        accum_out=sumx[:, h:h + 1] if act_stats else None)
                # ---- leaf BN stats for this group ----
                if act_stats:
                    # sumsq via ACT Square+accumulate; mean/var on gpsimd
                    sqs = sp.tile([80, 2 * BT], bf16, tag="sqs",
                                  name=f"sqs{gi}")
                    for h in range(2):
                        nc.scalar.activation(
                            sqs[:, :], ltile[:, 2 * h * BT:2 * (h + 1) * BT],
                            AF.Square, bias=zbias[:, 0:1],
                            accum_out=sumq[:, h:h + 1])
                    sxr = sp.tile([80, 1], f32, tag="sxr", name=f"sxr{gi}")
                    nc.vector.tensor_tensor(sxr, sumx[:, 0:1], sumx[:, 1:2],
                                            ALU.add)
                    nc.vector.tensor_scalar(mv_cols[:, gi, 0:1], sxr,
                                            1.0 / B, None, ALU.mult)
                    mq = sp.tile([80, 1], f32, tag="mq2", name=f"mq2{gi}")
                    nc.vector.tensor_tensor(mq, mv_cols[:, gi, 0:1],
                                            mv_cols[:, gi, 0:1], ALU.mult)
                    sqr = sp.tile([80, 1], f32, tag="sqr", name=f"sqr{gi}")
                    nc.vector.tensor_tensor(sqr, sumq[:, 0:1], sumq[:, 1:2],
                                            ALU.add)
                    nc.vector.tensor_scalar(mv_cols[:, gi, 1:2], sqr,
                                            1.0 / B, mq, ALU.mult,
                                            ALU.subtract)
                else:
                    st = sp.tile([80, NBT, 6], f32, tag="st", name=f"st{gi}")
                    for bt in range(NBT):
                        nc.vector.bn_stats(
                            out=st[:, bt, :],
                            in_=ltile[:, bt * BT:(bt + 1) * BT])
                    nc.vector.bn_aggr(out=mv_cols[:, gi, :], in_=st[:, :, :])

                if gi == 2:
                    # load mid gene inputs + start mid psum accumulation
                    # (placed here so the big DMA doesn't delay early leaves
                    # and the PE queue isn't blocked waiting for it)
                    xmid2_sb[0] = cp.tile_from(xmid2[0], name="xmid2a")
                    xmid2_sb[1] = cp.tile_from(xmid2[1], name="xmid2b")
                    for bt in range(NBT):
                        for pr in range(2):
                            nc.tensor.matmul(
                                mid_ps[bt][:, :],
                                wxmid2_sb[:, 80 * pr:80 * pr + 80],
                                xmid2_sb[pr][:, bt * BT:(bt + 1) * BT],
                                start=(pr == 0), stop=False)

                if gj != 3:
                    continue
                # ---- mid mi: leaf BN fold -> apply -> child matmuls ----
                mean4 = mv_cols[:, 4 * mi:4 * mi + 4, 0]
                var4 = mv_cols[:, 4 * mi:4 * mi + 4, 1]
                feng = nc.vector
                a4 = sp.tile([80, 4], f32, tag="a4", name=f"a4{mi}")
                feng.tensor_scalar(a4, var4, EPS, None, ALU.add)
                rs4 = sp.tile([80, 4], f32, tag="rs4", name=f"rs4{mi}")
                _emit_rsqrt(nc, feng, sp, "lf", rs4, a4, magic_t, 4)
                s_t = sp.tile([80, 4], f32, tag="s", name=f"s{mi}")
                feng.tensor_tensor(s_t, gleaf_sb[:, 4 * mi:4 * mi + 4], rs4,
                                   ALU.mult)
                ms = sp.tile([80, 4], f32, tag="ms", name=f"ms{mi}")
                feng.tensor_tensor(ms, mean4, s_t, ALU.mult)
                t_t = sp.tile([80, 4], f32, tag="t", name=f"t{mi}")
                feng.tensor_tensor(t_t, beleaf_sb[:, 4 * mi:4 * mi + 4], ms,
                                   ALU.subtract)

                # apply BN in place on the 4 bf16 leaf tiles
                for g2 in range(4):
                    idx = 4 * mi + g2
                    nc.vector.tensor_scalar(
                        lt_tiles[idx][:, :], lt_tiles[idx][:, :],
                        s_t[:, g2:g2 + 1], t_t[:, g2:g2 + 1],
                        ALU.mult, ALU.add)
                for bt in range(NBT):
                    for g2 in range(4):
                        idx = 4 * mi + g2
                        nc.tensor.matmul(
                            mid_ps[bt][:, :],
                            wgmid_sb[:, 80 * idx:80 * idx + 80],
                            lt_tiles[idx][:, bt * BT:(bt + 1) * BT],
                            start=False, stop=(idx == GPC - 1))

            # ---- mid finish: tanh into stack, BN stats, fold into root ----
            mst = sp.tile([80, NBT, 6], f32, tag="mst", name="mst")
            for bt in range(NBT):
                nc.scalar.activation(
                    stack[0:80, bt * BT:(bt + 1) * BT], mid_ps[bt][:, :],
                    AF.Tanh, bias=bmid_sb[:, 0:1], scale=1.0)
                nc.vector.bn_stats(out=mst[:, bt, :],
                                   in_=stack[0:80, bt * BT:(bt + 1) * BT])
            mmv = sp.tile([80, 2], f32, tag="mmv", name="mmv")
            nc.vector.bn_aggr(out=mmv[:, :], in_=mst[:, :, :])
            ma = sp.tile([80, 1], f32, tag="ma", name="ma")
            nc.vector.tensor_scalar_add(ma, mmv[:, 1:2], EPS)
            mrs = sp.tile([80, 1], f32, tag="mrs", name="mrs")
            _emit_rsqrt(nc, nc.vector, sp, "md", mrs, ma, magic_t, 1)
            msm = sp.tile([80, 1], f32, tag="msm", name="msm")
            nc.vector.tensor_mul(msm, gmid_sb[:, :], mrs)
            mms = sp.tile([80, 1], f32, tag="mms", name="mms")
            nc.vector.tensor_mul(mms, mmv[:, 0:1], msm)
            mtm = sp.tile([80, 1], f32, tag="mtm", name="mtm")
            nc.vector.tensor_sub(mtm, bemid_sb[:, :], mms)
            # fold mid BN: scale wcroot rows; offset row via tiny matmul
            nc.vector.tensor_scalar(wrt[0:80, :], wcroot_sb[:, :],
                                    msm[:, 0:1], None, ALU.mult)
            tmid_bf = sp.tile([80, 1], bf16, tag="tmbf", name="tmidbf")
            nc.vector.tensor_copy(tmid_bf, mtm)
            pr_ps = psA.tile([1, OR], f32, tag="leaf", name="prps")
            nc.tensor.matmul(pr_ps[:, :], tmid_bf[:, 0:1], wcroot_sb[:, :],
                             start=True, stop=True)
            nc.vector.tensor_tensor(wrt[96:97, :], pr_ps[:, :],
                                    broot8_sb[0:1, :], ALU.add)

            # ---- root partial in wire layout [128, 16*38] ----
            wire_ps = psA.tile([128, 2 * BT], f32, tag="leaf", name="wireps")
            for c in range(NCH):
                col = 38 * c if c < 13 else BT + 38 * (c - 13)
                nc.tensor.matmul(
                    wire_ps[:, col:col + OR],
                    stack[:, 128 * c:128 * (c + 1)],
                    wrt[:, :], start=True, stop=True)
            wire_sb = bp.tile([128, NCH * OR], bf16, tag="wire",
                              name="wiresb")
            nc.vector.tensor_copy(wire_sb[:, 0:13 * OR],
                                  wire_ps[:, 0:13 * OR])
            nc.vector.tensor_copy(wire_sb[:, 13 * OR:NCH * OR],
                                  wire_ps[:, BT:BT + 3 * OR])

            # ---- AllReduce the partial root pre-activation (bf16) ----
            cc_in = dp.tile([128, NCH * OR], bf16, tag="ccin", name="ccin")
            cc_out = dp.tile([128, NCH * OR], bf16, tag="ccout",
                             name="ccout", addr_space="Shared")
            nc.sync.dma_start(out=cc_in[:, :], in_=wire_sb[:, :])
            nc.gpsimd.collective_compute(
                "AllReduce",
                ALU.add,
                replica_groups=[list(range(NCORES))],
                ins=[cc_in.opt()],
                outs=[cc_out.opt()],
            )
            rsum = bp.tile([128, NCH * OR], bf16, tag="rsum", name="rsum")
            nc.sync.dma_start(out=rsum[:, :], in_=cc_out[:, :])

            # ---- root finish: tanh, full-batch BN in wire layout ----
            rt = bp.tile([128, NCH * OR], bf16, tag="rt", name="rt")
            nc.scalar.activation(rt[:, :], rsum[:, :], AF.Tanh,
                                 bias=0.0, scale=1.0)
            sq = bp.tile([128, NCH * OR], bf16, tag="rsq", name="rsq")
            nc.vector.tensor_tensor(sq[:, :], rt[:, :], rt[:, :], ALU.mult)
            rt_jc = rt[:, :].rearrange("p (c j) -> p j c", c=NCH)
            sq_jc = sq[:, :].rearrange("p (c j) -> p j c", c=NCH)
            rs38 = sp.tile([128, OR], f32, tag="rs38", name="rs38")
            nc.vector.tensor_reduce(out=rs38, in_=rt_jc,
                                    op=ALU.add, axis=AX.X)
            qs38 = sp.tile([128, OR], f32, tag="qs38", name="qs38")
            nc.vector.tensor_reduce(out=qs38, in_=sq_jc,
                                    op=ALU.add, axis=AX.X)
            import concourse.bass_isa as bass_isa
            S_all = sp.tile([128, OR], f32, tag="Sall", name="Sall")
            nc.gpsimd.partition_all_reduce(S_all[:, :], rs38[:, :],
                                           channels=128,
                                           reduce_op=bass_isa.ReduceOp.add)
            Q_all = sp.tile([128, OR], f32, tag="Qall", name="Qall")
            nc.gpsimd.partition_all_reduce(Q_all[:, :], qs38[:, :],
                                           channels=128,
                                           reduce_op=bass_isa.ReduceOp.add)
            mean_t = sp.tile([128, OR], f32, tag="rmean", name="rmean")
            nc.vector.tensor_scalar(mean_t, S_all, 1.0 / B, None, ALU.mult)
            msq_t = sp.tile([128, OR], f32, tag="rmsq", name="rmsq")
            nc.vector.tensor_tensor(msq_t, mean_t, mean_t, ALU.mult)
            qb_t = sp.tile([128, OR], f32, tag="rqb", name="rqb")
            nc.vector.tensor_scalar(qb_t, Q_all, 1.0 / B, EPS, ALU.mult,
                                    ALU.add)
            va_t = sp.tile([128, OR], f32, tag="rva", name="rva")
            nc.vector.tensor_tensor(va_t, qb_t, msq_t, ALU.subtract)
            rrs = sp.tile([128, OR], f32, tag="rrs", name="rrs")
            _emit_rsqrt(nc, nc.vector, sp, "rt", rrs, va_t, magic_r, OR)
            rsc = sp.tile([128, OR], f32, tag="rsc", name="rsc")
            nc.vector.tensor_tensor(rsc, grootb_sb[:, :], rrs, ALU.mult)
            rmsh = sp.tile([128, OR], f32, tag="rmsh", name="rmsh")
            nc.vector.tensor_tensor(rmsh, mean_t, rsc, ALU.mult)
            rsh = sp.tile([128, OR], f32, tag="rsh", name="rsh")
            nc.vector.tensor_tensor(rsh, berootb_sb[:, :], rmsh,
                                    ALU.subtract)
            # y = rt * scale + shift (scale/shift broadcast over chunks)
            sc_b = rsc[:, :].unsqueeze(1).broadcast_to([128, NCH, OR])
            sh_b = rsh[:, :].unsqueeze(1).broadcast_to([128, NCH, OR])
            rt_cj = rt[:, :].rearrange("p (c j) -> p c j", c=NCH)
            tmp = bp.tile([128, NCH * OR], f32, tag="tmp", name="tmpn")
            nc.vector.tensor_tensor(
                tmp[:, :].rearrange("p (c j) -> p c j", c=NCH),
                rt_cj, sc_b, ALU.mult)
            ysb = bp.tile([128, NCH * OR], f32, tag="ysb", name="ysb")
            nc.vector.tensor_tensor(
                ysb[:, :].rearrange("p (c j) -> p c j", c=NCH),
                tmp[:, :].rearrange("p (c j) -> p c j", c=NCH),
                sh_b, ALU.add)
            nc.sync.dma_start(out=y, in_=ysb[:, :])

    nc.compile()
    return nc


def _prep_in_maps(inputs):
    """Host-side sharding + layout prep (incl. bf16 cast). 8 in_maps."""
    f = np.float32
    x_leaf = np.asarray(inputs["x_leaf"], dtype=f)
    x_mid = np.asarray(inputs["x_mid"], dtype=f)
    x_root = np.asarray(inputs["x_root"], dtype=f)
    W_leaf = np.asarray(inputs["W_leaf"], dtype=f)
    b_leaf = np.asarray(inputs["b_leaf"], dtype=f)
    g_leaf = np.asarray(inputs["g_leaf"], dtype=f)
    be_leaf = np.asarray(inputs["be_leaf"], dtype=f)
    W_mid = np.asarray(inputs["W_mid"], dtype=f)
    b_mid = np.asarray(inputs["b_mid"], dtype=f)
    g_mid = np.asarray(inputs["g_mid"], dtype=f)
    be_mid = np.asarray(inputs["be_mid"], dtype=f)
    W_root = np.asarray(inputs["W_root"], dtype=f)
    b_root = np.asarray(inputs["b_root"], dtype=f)
    g_root = np.asarray(inputs["g_root"], dtype=f)
    be_root = np.asarray(inputs["be_root"], dtype=f)

    # gene-major leaf inputs, 4 leaves stacked per 128-partition group
    xleafT = np.ascontiguousarray(
        x_leaf.reshape(NCORES, GPC, 4, B, GL).transpose(0, 1, 2, 4, 3)
        .reshape(NCORES, GPC, 128, B)).astype(NPBF16)
    # mid gene inputs: per core, mid pairs (0,1) and (2,3) stacked to 128
    xmidT = (x_mid.reshape(NCORES, 2, 2, B, GM).transpose(0, 1, 2, 4, 3)
             .reshape(NCORES, 2, 128, B)).astype(NPBF16)
    xrootT = np.ascontiguousarray(x_root.T).astype(NPBF16)     # [128, B]

    in_maps = []
    for c in range(NCORES):
        d = {}
        d["xleaf"] = np.ascontiguousarray(xleafT[c])
        # block-diagonal leaf weights [128, 16*80]
        wl = np.zeros((128, GPC * 80), f)
        for gi in range(GPC):
            for j in range(4):
                s = LPC * c + 4 * gi + j
                wl[32 * j:32 * j + 32,
                   80 * gi + 20 * j:80 * gi + 20 * j + 20] = W_leaf[s]
        d["wleaf"] = wl.astype(NPBF16)
        for src, name in ((b_leaf, "bleaf"), (g_leaf, "gleaf"),
                          (be_leaf, "beleaf")):
            d[name] = np.ascontiguousarray(
                src[LPC * c:LPC * (c + 1)].reshape(GPC, 80).T)
        d["xmid2"] = np.ascontiguousarray(xmidT[c])
        wg = np.zeros((80, GPC * 80), f)
        # gene blocks for mid pairs: [128, 2*80]
        wx2 = np.zeros((128, 2 * 80), f)
        for mi in range(MPC):
            m = MPC * c + mi
            for gj in range(4):
                idx = 4 * mi + gj
                wg[:, 80 * idx + 20 * mi:80 * idx + 20 * mi + 20] = \
                    W_mid[m, GM + 80 * gj:GM + 80 * gj + 80, :]
            pr, sub = mi // 2, mi % 2
            wx2[64 * sub:64 * sub + 64,
                80 * pr + 20 * mi:80 * pr + 20 * mi + 20] = W_mid[m, :GM, :]
        d["wgmid"] = wg.astype(NPBF16)
        d["wxmid2"] = wx2.astype(NPBF16)
        for src, name in ((b_mid, "bmid"), (g_mid, "gmid"), (be_mid, "bemid")):
            d[name] = np.ascontiguousarray(
                src[MPC * c:MPC * (c + 1)].reshape(80, 1))
        d["wcroot"] = np.ascontiguousarray(
            W_root[GR + 80 * c:GR + 80 * (c + 1), :]).astype(NPBF16)
        d["wgroot"] = np.ascontiguousarray(
            W_root[16 * c:16 * (c + 1), :]).astype(NPBF16)
        x17 = np.ones((17, B), f)
        x17[0:16, :] = xrootT[16 * c:16 * (c + 1), :].astype(f)
        d["xroot17"] = x17.astype(NPBF16)
        d["broot8"] = np.ascontiguousarray(
            (b_root / NCORES).reshape(1, OR))
        d["grootb"] = np.ascontiguousarray(
            np.broadcast_to(g_root.reshape(1, OR), (128, OR)))
        d["berootb"] = np.ascontiguousarray(
            np.broadcast_to(be_root.reshape(1, OR), (128, OR)))
        in_maps.append(d)
    return in_maps


_NC_CACHE = {}


def _get_nc():
    if "nc" not in _NC_CACHE:
        _NC_CACHE["nc"] = _build_nc()
    return _NC_CACHE["nc"]


def _postprocess(y_dev) -> np.ndarray:
    """[128, 16*38] wire-layout device output -> [2048, 38] float32."""
    out = np.asarray(y_dev, dtype=np.float32).reshape(128, NCH, OR)
    return np.ascontiguousarray(out.transpose(1, 0, 2).reshape(B, OR))


def kernel(**inputs) -> np.ndarray:
    nc = _get_nc()
    in_maps = _prep_in_maps(inputs)
    res = bass_utils.run_bass_kernel_spmd(
        nc, in_maps, core_ids=list(range(NCORES)))
    return _postprocess(res.results[0]["y"])


# revision 12
# speedup vs baseline: 1.2071x; 1.0997x over previous
"""Trainium2 Bass kernel for nn_DCell (hierarchical DCell-style GNN).

Sharding: subsystem-parallel across 8 NeuronCores. Each core owns 64 of the
512 leaf subsystems (16 groups of 4 leaves, block-diagonal matmuls with
K=128) and the 4 mid subsystems fed by exactly those leaves. BatchNorm batch
stats are fully local for leaf and mid layers. The mid BN affine is folded
into the root weights; each core computes its root partial pre-activation in
a transposed [batch-on-partitions] wire layout [128, 16*38], which is summed
across cores with one AllReduce. Every core then redundantly finishes the
root (tanh + full-batch BN via a gpsimd partition-reduce) and writes the
[128, 608] f32 result; the host reassembles [2048, 38].

Work is spread across all four compute engines: PE (matmuls), ACT (tanh +
some stats via Square-accumulate), DVE (bn_stats, folds, finish), Pool/
GpSimd (BN applies, small stat reductions, partition reduce/broadcast).

kernel(**inputs) takes full unsharded inputs, returns [2048, 38] float32.
"""

import ml_dtypes
import numpy as np

import concourse.bass as bass
import concourse.mybir as mybir
import concourse.tile as tile
from concourse import bacc
from concourse import bass_utils

# Problem constants (hardcoded; kernel.py must be self-contained)
S, B, GL, OL = 512, 2048, 32, 20
M, C, GM, OM = 32, 16, 64, 20
GR, OR = 128, 38
EPS = 1e-5
NCORES = 8
LPC = S // NCORES      # 64 leaves per core
GPC = LPC // 4         # 16 leaf groups of 4 per core
MPC = M // NCORES      # 4 mids per core
BT = 512               # batch tile (free dim per matmul / psum bank)
NBT = B // BT          # 4
NCH = B // 128         # 16 batch chunks of 128 (wire layout)

f32 = mybir.dt.float32
bf16 = mybir.dt.bfloat16
i32 = mybir.dt.int32
AF = mybir.ActivationFunctionType
ALU = mybir.AluOpType
AX = mybir.AxisListType
NPBF16 = ml_dtypes.bfloat16

MAGIC = 0x5F3759DF  # fast inverse sqrt seed

# stats-engine assignment per leaf group: these groups compute sumsq via
# ACT Square+accumulate, the rest via DVE bn_stats
ACT_STATS = {4, 5, 6, 7, 8, 9, 10}


def _emit_rsqrt(nc, eng, sp, tag, out, a, magic_t, n):
    """out = 1/sqrt(a) elementwise on [P, n] fp32 tiles, DVE/Pool-only.

    Quake magic seed + 2 Newton iterations: rel err ~5e-6. a > 0.
    """
    P = a.shape[0]
    sh = sp.tile([P, n], i32, tag=f"{tag}sh", name=f"{tag}sh")
    eng.tensor_scalar(sh, a.bitcast(i32), 1, None, ALU.arith_shift_right)
    y0 = sp.tile([P, n], i32, tag=f"{tag}y0", name=f"{tag}y0")
    eng.tensor_tensor(y0, magic_t[:P, 0:n], sh, ALU.subtract)
    y = y0.bitcast(f32)
    for it in range(2):
        # y <- y * (1.5 - 0.5*a*y*y)
        t1 = sp.tile([P, n], f32, tag=f"{tag}t1", name=f"{tag}t1_{it}")
        eng.tensor_tensor(t1, y, y, ALU.mult)                # y^2
        t2 = sp.tile([P, n], f32, tag=f"{tag}t2", name=f"{tag}t2_{it}")
        eng.tensor_tensor(t2, a, t1, ALU.mult)               # a*y^2
        t3 = sp.tile([P, n], f32, tag=f"{tag}t3", name=f"{tag}t3_{it}")
        eng.tensor_scalar(t3, t2, -0.5, 1.5, ALU.mult, ALU.add)
        dst = out if it == 1 else sp.tile([P, n], f32, tag=f"{tag}y",
                                          name=f"{tag}y_{it}")
        eng.tensor_tensor(dst, y, t3, ALU.mult)
        y = dst


def _build_nc():
    """Build (once) the SPMD Bass program run identically on all 8 cores."""
    nc = bacc.Bacc(
        "TRN2",
        target_bir_lowering=False,
        debug=False,
        enable_asserts=False,
        num_devices=NCORES,
    )

    # ---- per-core external I/O ----
    xleaf = nc.dram_tensor("xleaf", [GPC, 128, B], bf16, kind="ExternalInput").ap()
    wleaf = nc.dram_tensor("wleaf", [128, GPC * 80], bf16, kind="ExternalInput").ap()
    bleaf = nc.dram_tensor("bleaf", [80, GPC], f32, kind="ExternalInput").ap()
    gleaf = nc.dram_tensor("gleaf", [80, GPC], f32, kind="ExternalInput").ap()
    beleaf = nc.dram_tensor("beleaf", [80, GPC], f32, kind="ExternalInput").ap()
    xmid2 = nc.dram_tensor("xmid2", [2, 128, B], bf16, kind="ExternalInput").ap()
    wgmid = nc.dram_tensor("wgmid", [80, GPC * 80], bf16, kind="ExternalInput").ap()
    wxmid2 = nc.dram_tensor("wxmid2", [128, 2 * 80], bf16, kind="ExternalInput").ap()
    bmid = nc.dram_tensor("bmid", [80, 1], f32, kind="ExternalInput").ap()
    gmid = nc.dram_tensor("gmid", [80, 1], f32, kind="ExternalInput").ap()
    bemid = nc.dram_tensor("bemid", [80, 1], f32, kind="ExternalInput").ap()
    wcroot = nc.dram_tensor("wcroot", [80, OR], bf16, kind="ExternalInput").ap()
    wgroot = nc.dram_tensor("wgroot", [16, OR], bf16, kind="ExternalInput").ap()
    xroot17 = nc.dram_tensor("xroot17", [17, B], bf16, kind="ExternalInput").ap()
    broot8 = nc.dram_tensor("broot8", [1, OR], f32, kind="ExternalInput").ap()
    grootb = nc.dram_tensor("grootb", [128, OR], f32, kind="ExternalInput").ap()
    berootb = nc.dram_tensor("berootb", [128, OR], f32, kind="ExternalInput").ap()
    y = nc.dram_tensor("y", [128, NCH * OR], f32, kind="ExternalOutput").ap()

    with tile.TileContext(nc) as tc:
        with (
            tc.tile_pool(name="const", bufs=1) as cp,
            tc.tile_pool(name="xp", bufs=3) as xp,
            tc.tile_pool(name="lt", bufs=16) as ltp,
            tc.tile_pool(name="small", bufs=2) as sp,
            tc.tile_pool(name="big", bufs=1) as bp,
            tc.tile_pool(name="psA", bufs=2, space="PSUM") as psA,
            tc.tile_pool(name="psM", bufs=4, space="PSUM") as psM,
            tc.tile_pool(name="dram", bufs=1, space="DRAM") as dp,
        ):
            # ---- load constants/weights into SBUF (xmid2 deferred) ----
            wleaf_sb = cp.tile_from(wleaf, forced_dma_engine=mybir.EngineType.Pool)
            bleaf_sb = cp.tile_from(bleaf, forced_dma_engine=mybir.EngineType.Pool)
            gleaf_sb = cp.tile_from(gleaf, forced_dma_engine=mybir.EngineType.Pool)
            beleaf_sb = cp.tile_from(beleaf, forced_dma_engine=mybir.EngineType.Pool)
            wgmid_sb = cp.tile_from(wgmid, forced_dma_engine=mybir.EngineType.Pool)
            wxmid2_sb = cp.tile_from(wxmid2, forced_dma_engine=mybir.EngineType.Pool)
            bmid_sb = cp.tile_from(bmid, forced_dma_engine=mybir.EngineType.Pool)
            gmid_sb = cp.tile_from(gmid, forced_dma_engine=mybir.EngineType.Pool)
            bemid_sb = cp.tile_from(bemid, forced_dma_engine=mybir.EngineType.Pool)
            wcroot_sb = cp.tile_from(wcroot, forced_dma_engine=mybir.EngineType.Pool)
            broot8_sb = cp.tile_from(broot8, forced_dma_engine=mybir.EngineType.Pool)
            grootb_sb = cp.tile_from(grootb, forced_dma_engine=mybir.EngineType.Pool)
            berootb_sb = cp.tile_from(berootb, forced_dma_engine=mybir.EngineType.Pool)

            # stacked root lhsT: rows 0-79 mid tanh out, 80-96 xroot+ones
            stack = bp.tile([97, B], bf16, tag="stack", name="stack")
            nc.gpsimd.dma_start(out=stack[80:97, :], in_=xroot17)
            # stacked root weights: 0-79 scaled wcroot, 80-95 wgroot,
            # 96 = t_mid fold + b_root/8
            wrt = bp.tile([97, OR], bf16, tag="wrt", name="wrt")
            nc.gpsimd.dma_start(out=wrt[80:96, :], in_=wgroot)

            magic_t = cp.tile([80, 4], i32, tag="magic", name="magict")
            nc.vector.memset(magic_t, MAGIC)
            magic_r = cp.tile([128, OR], i32, tag="magicr", name="magicr")
            nc.vector.memset(magic_r, MAGIC)
            zbias = cp.tile([80, 1], f32, tag="zbias", name="zbias")
            nc.vector.memset(zbias, 0.0)

            # persistent mid-accumulation psum banks (one per batch tile)
            mid_ps = [psM.tile([80, BT], f32, tag="mid", name=f"midps{b}")
                      for b in range(NBT)]

            lt_tiles = []
            mv_cols = cp.tile([80, GPC, 2], f32, tag="mvall", name="mvall")
            xmid2_sb = [None, None]
            for gi in range(GPC):
                mi, gj = gi // 4, gi % 4
                act_stats = gi in ACT_STATS
                ltile = ltp.tile([80, B], bf16, tag="lt", name=f"lt{gi}")
                lt_tiles.append(ltile)
                if act_stats:
                    sumx = sp.tile([80, 2], f32, tag="sx", name=f"sx{gi}")
                    sumq = sp.tile([80, 2], f32, tag="sq", name=f"sq{gi}")
                # ---- leaf matmul + tanh (+ Sum accum) per half batch ----
                xt = xp.tile([128, B], bf16, tag="x", name=f"x{gi}")
                nc.sync.dma_start(out=xt, in_=xleaf[gi])
                for h in range(2):
                    ps = psA.tile([80, 2 * BT], f32, tag="leaf",
                                  name=f"lfps{gi}_{h}")
                    for s2 in range(2):
                        nc.tensor.matmul(
                            ps[:, s2 * BT:(s2 + 1) * BT],
                            wleaf_sb[:, 80 * gi:80 * gi + 80],
                            xt[:, (2 * h + s2) * BT:(2 * h + s2 + 1) * BT],
                            start=True, stop=True)
                    nc.scalar.activation(
                        ltile[:, 2 * h * BT:2 * (h + 1) * BT], ps[:, :],
                        AF.Tanh, bias=bleaf_sb[:, gi:gi + 1], scale=1.0,
                        accum_out=sumx[:, h:h + 1] if act_stats else None)
                # ---- leaf BN stats for this group ----
                if act_stats:
                    # sumsq via ACT Square+accumulate; mean/var on gpsimd
                    sqs = sp.tile([80, 2 * BT], bf16, tag="sqs",
                                  name=f"sqs{gi}")
                    for h in range(2):
                        nc.scalar.activation(
                            sqs[:, :], ltile[:, 2 * h * BT:2 * (h + 1) * BT],
                            AF.Square, bias=zbias[:, 0:1],
                            accum_out=sumq[:, h:h + 1])
                    sxr = sp.tile([80, 1], f32, tag="sxr", name=f"sxr{gi}")
                    nc.vector.tensor_tensor(sxr, sumx[:, 0:1], sumx[:, 1:2],
                                            ALU.add)
                    nc.vector.tensor_scalar(mv_cols[:, gi, 0:1], sxr,
                                            1.0 / B, None, ALU.mult)
                    mq = sp.tile([80, 1], f32, tag="mq2", name=f"mq2{gi}")
                    nc.vector.tensor_tensor(mq, mv_cols[:, gi, 0:1],
                                            mv_cols[:, gi, 0:1], ALU.mult)
                    sqr = sp.tile([80, 1], f32, tag="sqr", name=f"sqr{gi}")
                    nc.vector.tensor_tensor(sqr, sumq[:, 0:1], sumq[:, 1:2],
                                            ALU.add)
                    nc.vector.tensor_scalar(mv_cols[:, gi, 1:2], sqr,
                                            1.0 / B, mq, ALU.mult,
                                            ALU.subtract)
                else:
                    st = sp.tile([80, NBT, 6], f32, tag="st", name=f"st{gi}")
                    for bt in range(NBT):
                        nc.vector.bn_stats(
                            out=st[:, bt, :],
                            in_=ltile[:, bt * BT:(bt + 1) * BT])
                    nc.vector.bn_aggr(out=mv_cols[:, gi, :], in_=st[:, :, :])

                if gi == 2:
                    # load mid gene inputs + start mid psum accumulation
                    # (placed here so the big DMA doesn't delay early leaves
                    # and the PE queue isn't blocked waiting for it)
                    xmid2_sb[0] = cp.tile_from(xmid2[0], name="xmid2a", forced_dma_engine=mybir.EngineType.Pool)
                    xmid2_sb[1] = cp.tile_from(xmid2[1], name="xmid2b", forced_dma_engine=mybir.EngineType.Pool)
                    for bt in range(NBT):
                        for pr in range(2):
                            nc.tensor.matmul(
                                mid_ps[bt][:, :],
                                wxmid2_sb[:, 80 * pr:80 * pr + 80],
                                xmid2_sb[pr][:, bt * BT:(bt + 1) * BT],
                                start=(pr == 0), stop=False)

                if gj != 3:
                    continue
                # ---- mid mi: leaf BN fold -> apply -> child matmuls ----
                mean4 = mv_cols[:, 4 * mi:4 * mi + 4, 0]
                var4 = mv_cols[:, 4 * mi:4 * mi + 4, 1]
                feng = nc.vector
                a4 = sp.tile([80, 4], f32, tag="a4", name=f"a4{mi}")
                feng.tensor_scalar(a4, var4, EPS, None, ALU.add)
                rs4 = sp.tile([80, 4], f32, tag="rs4", name=f"rs4{mi}")
                _emit_rsqrt(nc, feng, sp, "lf", rs4, a4, magic_t, 4)
                s_t = sp.tile([80, 4], f32, tag="s", name=f"s{mi}")
                feng.tensor_tensor(s_t, gleaf_sb[:, 4 * mi:4 * mi + 4], rs4,
                                   ALU.mult)
                ms = sp.tile([80, 4], f32, tag="ms", name=f"ms{mi}")
                feng.tensor_tensor(ms, mean4, s_t, ALU.mult)
                t_t = sp.tile([80, 4], f32, tag="t", name=f"t{mi}")
                feng.tensor_tensor(t_t, beleaf_sb[:, 4 * mi:4 * mi + 4], ms,
                                   ALU.subtract)

                # apply BN in place on the 4 bf16 leaf tiles
                for g2 in range(4):
                    idx = 4 * mi + g2
                    nc.vector.tensor_scalar(
                        lt_tiles[idx][:, :], lt_tiles[idx][:, :],
                        s_t[:, g2:g2 + 1], t_t[:, g2:g2 + 1],
                        ALU.mult, ALU.add)
                for g2 in range(4):
                    idx = 4 * mi + g2
                    for bt in range(NBT):
                        nc.tensor.matmul(
                            mid_ps[bt][:, :],
                            wgmid_sb[:, 80 * idx:80 * idx + 80],
                            lt_tiles[idx][:, bt * BT:(bt + 1) * BT],
                            start=False, stop=(idx == GPC - 1))

            # ---- mid finish: tanh into stack, BN stats, fold into root ----
            mst = sp.tile([80, NBT, 6], f32, tag="mst", name="mst")
            for bt in range(NBT):
                nc.scalar.activation(
                    stack[0:80, bt * BT:(bt + 1) * BT], mid_ps[bt][:, :],
                    AF.Tanh, bias=bmid_sb[:, 0:1], scale=1.0)
                nc.vector.bn_stats(out=mst[:, bt, :],
                                   in_=stack[0:80, bt * BT:(bt + 1) * BT])
            mmv = sp.tile([80, 2], f32, tag="mmv", name="mmv")
            nc.vector.bn_aggr(out=mmv[:, :], in_=mst[:, :, :])
            ma = sp.tile([80, 1], f32, tag="ma", name="ma")
            nc.vector.tensor_scalar_add(ma, mmv[:, 1:2], EPS)
            mrs = sp.tile([80, 1], f32, tag="mrs", name="mrs")
            _emit_rsqrt(nc, nc.vector, sp, "md", mrs, ma, magic_t, 1)
            msm = sp.tile([80, 1], f32, tag="msm", name="msm")
            nc.vector.tensor_mul(msm, gmid_sb[:, :], mrs)
            mms = sp.tile([80, 1], f32, tag="mms", name="mms")
            nc.vector.tensor_mul(mms, mmv[:, 0:1], msm)
            mtm = sp.tile([80, 1], f32, tag="mtm", name="mtm")
            nc.vector.tensor_sub(mtm, bemid_sb[:, :], mms)
            # fold mid BN: scale wcroot rows; offset row via tiny matmul
            nc.vector.tensor_scalar(wrt[0:80, :], wcroot_sb[:, :],
                                    msm[:, 0:1], None, ALU.mult)
            tmid_bf = sp.tile([80, 1], bf16, tag="tmbf", name="tmidbf")
            nc.vector.tensor_copy(tmid_bf, mtm)
            pr_ps = psA.tile([1, OR], f32, tag="leaf", name="prps")
            nc.tensor.matmul(pr_ps[:, :], tmid_bf[:, 0:1], wcroot_sb[:, :],
                             start=True, stop=True)
            nc.vector.tensor_tensor(wrt[96:97, :], pr_ps[:, :],
                                    broot8_sb[0:1, :], ALU.add)

            # ---- root partial in wire layout [128, 16*38] ----
            wire_ps = psA.tile([128, 2 * BT], f32, tag="leaf", name="wireps")
            for c in range(NCH):
                col = 38 * c if c < 13 else BT + 38 * (c - 13)
                nc.tensor.matmul(
                    wire_ps[:, col:col + OR],
                    stack[:, 128 * c:128 * (c + 1)],
                    wrt[:, :], start=True, stop=True)
            wire_sb = bp.tile([128, NCH * OR], bf16, tag="wire",
                              name="wiresb")
            nc.vector.tensor_copy(wire_sb[:, 0:13 * OR],
                                  wire_ps[:, 0:13 * OR])
            nc.vector.tensor_copy(wire_sb[:, 13 * OR:NCH * OR],
                                  wire_ps[:, BT:BT + 3 * OR])

            # ---- AllReduce the partial root pre-activation (bf16) ----
            cc_in = dp.tile([128, NCH * OR], bf16, tag="ccin", name="ccin")
            cc_out = dp.tile([128, NCH * OR], bf16, tag="ccout",
                             name="ccout", addr_space="Shared")
            nc.sync.dma_start(out=cc_in[:, :], in_=wire_sb[:, :])
            nc.gpsimd.collective_compute(
                "AllReduce",
                ALU.add,
                replica_groups=[list(range(NCORES))],
                ins=[cc_in.opt()],
                outs=[cc_out.opt()],
            )
            rsum = bp.tile([128, NCH * OR], bf16, tag="rsum", name="rsum")
            nc.sync.dma_start(out=rsum[:, :], in_=cc_out[:, :])

            # ---- root finish: tanh, full-batch BN in wire layout ----
            rt = bp.tile([128, NCH * OR], bf16, tag="rt", name="rt")
            nc.scalar.activation(rt[:, :], rsum[:, :], AF.Tanh,
                                 bias=0.0, scale=1.0)
            sq = bp.tile([128, NCH * OR], bf16, tag="rsq", name="rsq")
            nc.scalar.activation(sq[:, :], rt[:, :], AF.Square,
                                 bias=0.0, scale=1.0)
            rt_jc = rt[:, :].rearrange("p (c j) -> p j c", c=NCH)
            sq_jc = sq[:, :].rearrange("p (c j) -> p j c", c=NCH)
            rs38 = sp.tile([128, OR], f32, tag="rs38", name="rs38")
            nc.vector.tensor_reduce(out=rs38, in_=rt_jc,
                                    op=ALU.add, axis=AX.X)
            qs38 = sp.tile([128, OR], f32, tag="qs38", name="qs38")
            nc.vector.tensor_reduce(out=qs38, in_=sq_jc,
                                    op=ALU.add, axis=AX.X)
            import concourse.bass_isa as bass_isa
            S_all = sp.tile([128, OR], f32, tag="Sall", name="Sall")
            nc.gpsimd.partition_all_reduce(S_all[:, :], rs38[:, :],
                                           channels=128,
                                           reduce_op=bass_isa.ReduceOp.add)
            Q_all = sp.tile([128, OR], f32, tag="Qall", name="Qall")
            nc.gpsimd.partition_all_reduce(Q_all[:, :], qs38[:, :],
                                           channels=128,
                                           reduce_op=bass_isa.ReduceOp.add)
            mean_t = sp.tile([128, OR], f32, tag="rmean", name="rmean")
            nc.vector.tensor_scalar(mean_t, S_all, 1.0 / B, None, ALU.mult)
            msq_t = sp.tile([128, OR], f32, tag="rmsq", name="rmsq")
            nc.vector.tensor_tensor(msq_t, mean_t, mean_t, ALU.mult)
            qb_t = sp.tile([128, OR], f32, tag="rqb", name="rqb")
            nc.vector.tensor_scalar(qb_t, Q_all, 1.0 / B, EPS, ALU.mult,
                                    ALU.add)
            va_t = sp.tile([128, OR], f32, tag="rva", name="rva")
            nc.vector.tensor_tensor(va_t, qb_t, msq_t, ALU.subtract)
            rrs = sp.tile([128, OR], f32, tag="rrs", name="rrs")
            _emit_rsqrt(nc, nc.vector, sp, "rt", rrs, va_t, magic_r, OR)
            rsc = sp.tile([128, OR], f32, tag="rsc", name="rsc")
            nc.vector.tensor_tensor(rsc, grootb_sb[:, :], rrs, ALU.mult)
            rmsh = sp.tile([128, OR], f32, tag="rmsh", name="rmsh")
            nc.vector.tensor_tensor(rmsh, mean_t, rsc, ALU.mult)
            rsh = sp.tile([128, OR], f32, tag="rsh", name="rsh")
            nc.vector.tensor_tensor(rsh, berootb_sb[:, :], rmsh,
                                    ALU.subtract)
            # y = rt * scale + shift (scale/shift broadcast over chunks)
            sc_b = rsc[:, :].unsqueeze(1).broadcast_to([128, NCH, OR])
            sh_b = rsh[:, :].unsqueeze(1).broadcast_to([128, NCH, OR])
            rt_cj = rt[:, :].rearrange("p (c j) -> p c j", c=NCH)
            tmp = bp.tile([128, NCH * OR], bf16, tag="tmp", name="tmpn")
            nc.vector.tensor_tensor(
                tmp[:, :].rearrange("p (c j) -> p c j", c=NCH),
                rt_cj, sc_b, ALU.mult)
            ysb = bp.tile([128, NCH * OR], f32, tag="ysb", name="ysb")
            nc.vector.tensor_tensor(
                ysb[:, :].rearrange("p (c j) -> p c j", c=NCH),
                tmp[:, :].rearrange("p (c j) -> p c j", c=NCH),
                sh_b, ALU.add)
            nc.gpsimd.dma_start(out=y, in_=ysb[:, :])

    nc.compile()
    return nc


def _prep_in_maps(inputs):
    """Host-side sharding + layout prep (incl. bf16 cast). 8 in_maps."""
    f = np.float32
    x_leaf = np.asarray(inputs["x_leaf"], dtype=f)
    x_mid = np.asarray(inputs["x_mid"], dtype=f)
    x_root = np.asarray(inputs["x_root"], dtype=f)
    W_leaf = np.asarray(inputs["W_leaf"], dtype=f)
    b_leaf = np.asarray(inputs["b_leaf"], dtype=f)
    g_leaf = np.asarray(inputs["g_leaf"], dtype=f)
    be_leaf = np.asarray(inputs["be_leaf"], dtype=f)
    W_mid = np.asarray(inputs["W_mid"], dtype=f)
    b_mid = np.asarray(inputs["b_mid"], dtype=f)
    g_mid = np.asarray(inputs["g_mid"], dtype=f)
    be_mid = np.asarray(inputs["be_mid"], dtype=f)
    W_root = np.asarray(inputs["W_root"], dtype=f)
    b_root = np.asarray(inputs["b_root"], dtype=f)
    g_root = np.asarray(inputs["g_root"], dtype=f)
    be_root = np.asarray(inputs["be_root"], dtype=f)

    # gene-major leaf inputs, 4 leaves stacked per 128-partition group
    xleafT = np.ascontiguousarray(
        x_leaf.reshape(NCORES, GPC, 4, B, GL).transpose(0, 1, 2, 4, 3)
        .reshape(NCORES, GPC, 128, B)).astype(NPBF16)
    # mid gene inputs: per core, mid pairs (0,1) and (2,3) stacked to 128
    xmidT = (x_mid.reshape(NCORES, 2, 2, B, GM).transpose(0, 1, 2, 4, 3)
             .reshape(NCORES, 2, 128, B)).astype(NPBF16)
    xrootT = np.ascontiguousarray(x_root.T).astype(NPBF16)     # [128, B]

    in_maps = []
    for c in range(NCORES):
        d = {}
        d["xleaf"] = np.ascontiguousarray(xleafT[c])
        # block-diagonal leaf weights [128, 16*80]
        wl = np.zeros((128, GPC * 80), f)
        for gi in range(GPC):
            for j in range(4):
                s = LPC * c + 4 * gi + j
                wl[32 * j:32 * j + 32,
                   80 * gi + 20 * j:80 * gi + 20 * j + 20] = W_leaf[s]
        d["wleaf"] = wl.astype(NPBF16)
        for src, name in ((b_leaf, "bleaf"), (g_leaf, "gleaf"),
                          (be_leaf, "beleaf")):
            d[name] = np.ascontiguousarray(
                src[LPC * c:LPC * (c + 1)].reshape(GPC, 80).T)
        d["xmid2"] = np.ascontiguousarray(xmidT[c])
        wg = np.zeros((80, GPC * 80), f)
        # gene blocks for mid pairs: [128, 2*80]
        wx2 = np.zeros((128, 2 * 80), f)
        for mi in range(MPC):
            m = MPC * c + mi
            for gj in range(4):
                idx = 4 * mi + gj
                wg[:, 80 * idx + 20 * mi:80 * idx + 20 * mi + 20] = \
                    W_mid[m, GM + 80 * gj:GM + 80 * gj + 80, :]
            pr, sub = mi // 2, mi % 2
            wx2[64 * sub:64 * sub + 64,
                80 * pr + 20 * mi:80 * pr + 20 * mi + 20] = W_mid[m, :GM, :]
        d["wgmid"] = wg.astype(NPBF16)
        d["wxmid2"] = wx2.astype(NPBF16)
        for src, name in ((b_mid, "bmid"), (g_mid, "gmid"), (be_mid, "bemid")):
            d[name] = np.ascontiguousarray(
                src[MPC * c:MPC * (c + 1)].reshape(80, 1))
        d["wcroot"] = np.ascontiguousarray(
            W_root[GR + 80 * c:GR + 80 * (c + 1), :]).astype(NPBF16)
        d["wgroot"] = np.ascontiguousarray(
            W_root[16 * c:16 * (c + 1), :]).astype(NPBF16)
        x17 = np.ones((17, B), f)
        x17[0:16, :] = xrootT[16 * c:16 * (c + 1), :].astype(f)
        d["xroot17"] = x17.astype(NPBF16)
        d["broot8"] = np.ascontiguousarray(
            (b_root / NCORES).reshape(1, OR))
        d["grootb"] = np.ascontiguousarray(
            np.broadcast_to(g_root.reshape(1, OR), (128, OR)))
        d["berootb"] = np.ascontiguousarray(
            np.broadcast_to(be_root.reshape(1, OR), (128, OR)))
        in_maps.append(d)
    return in_maps


_NC_CACHE = {}


def _get_nc():
    if "nc" not in _NC_CACHE:
        _NC_CACHE["nc"] = _build_nc()
    return _NC_CACHE["nc"]


def _postprocess(y_dev) -> np.ndarray:
    """[128, 16*38] wire-layout device output -> [2048, 38] float32."""
    out = np.asarray(y_dev, dtype=np.float32).reshape(128, NCH, OR)
    return np.ascontiguousarray(out.transpose(1, 0, 2).reshape(B, OR))


def kernel(**inputs) -> np.ndarray:
    nc = _get_nc()
    in_maps = _prep_in_maps(inputs)
    res = bass_utils.run_bass_kernel_spmd(
        nc, in_maps, core_ids=list(range(NCORES)))
    return _postprocess(res.results[0]["y"])
